# revision 16
# baseline (speedup 1.0000x reference)
"""DFlashAttention Trainium2 kernel (8-core tensor-parallel over attention heads).

Shapes (hardcoded): D=2048, N=16 q-heads, K=8 kv-heads, H=128,
T_NOISE=2048 (query tokens), T_CTX=4096, S=6144 (kv tokens).

Sharding: core c owns q-heads {2c, 2c+1} and kv-head c (GQA groups=2).
Each core computes a partial (T, D) output (its 2 heads' slice of the
o-projection contraction); the host sums the 8 partials (TP unshard).

v4 layout strategy per core (all matmul operands bf16, PSUM fp32):
  - x_all^T [D, S] fed bf16; ONE big DMA per 512-token chunk, first two
    chunks prefetched before every other constant load so the PE starts
    within ~10 us.
  - fused QKV projection: one weight tile [wk|wv|wq0|wq1] per d-tile;
    noise-token chunks project q in the same matmul group as k/v
    (512-wide moving operand), so x is loaded exactly once. Chunks are
    emitted noise/ctx interleaved and each chunk's transposes are
    deferred behind the next chunk's matmuls (in-order PE queue hazard).
  - RMSNorm: per-chunk batched stats (Squares w/ accum, one Sqrt, one
    reciprocal); the normalize copy runs on ScalarE (Copy w/ per-token
    scale); the rms scale vector is folded into HOST-precomputed
    rope tables (cA=cos*s1, sA=sin*s2, cB=cos*s2, sB=sin*s1), and the
    rope multiplies run 4-tiles-at-a-time on strided APs.
  - attention in [s, t] orientation, software-pipelined: scores^T for
    pair i+1 are issued before A@V of pair i so the PE streams through
    exp latency. exp on ACT straight to bf16; no max subtraction.
    Row-sums: exp tiles are quad-summed on the (idle) vector engine and
    a single ones-matmul per 4 s-tiles accumulates the denominators
    (third fewer TensorE instructions in the attention inner loop).
  - softmax normalization applied at the oT copy: row-sums -> SBUF ->
    GpSimd partition_broadcast -> reciprocal -> one tensor_mul.
    Phase D is a pure 2-head accumulating o-projection, PSUM evacuated
    alternately by ScalarE/VectorE, one 1 MB store per t-tile.
"""

import sys

for _p in ("/opt/trn_rl_repo", "/root/.axon_site/_ro/trn_rl_repo"):
    if _p not in sys.path:
        sys.path.append(_p)

import math
import numpy as np
import ml_dtypes

import concourse.bass as bass
import concourse.tile as tile
from concourse import bacc
from concourse import mybir
from concourse.bass_utils import run_bass_kernel_spmd
from concourse.masks import make_identity

D = 2048
N_HEADS = 16
K_HEADS = 8
H = 128
T_NOISE = 2048
T_CTX = 4096
S_ALL = T_CTX + T_NOISE          # 6144
EPS = 1e-6
ROPE_THETA = 1e6
N_CORES = 8
HEADS_PER_CORE = N_HEADS // N_CORES   # 2

P = 128                       # partition dim
S_TILES = S_ALL // P          # 48
T_TILES = T_NOISE // P        # 16
NOISE_TILE0 = T_CTX // P      # 32  (noise tokens are s-tiles 32..47)
D_TILES = D // P              # 16
FREE = 512                    # moving free-dim chunk
T_CHUNKS = T_NOISE // FREE    # 4
S_CHUNKS = S_ALL // FREE      # 12
D_CHUNKS = D // FREE          # 4
NOISE_CHUNK0 = T_CTX // FREE  # 8   (chunks 8..11 are noise tokens)
HALF = H // 2

F32 = mybir.dt.float32
BF16 = mybir.dt.bfloat16
MM_DT = BF16                  # dtype for all matmul operands

INV_SQRT_H = 1.0 / math.sqrt(H)
MULT = mybir.AluOpType.mult

_CACHE = {}

# phase-A chunk emission order: noise/ctx interleaved so adjacent chunks
# never need two full noise PSUM slot sets at once, and the PE always has
# a fresh chunk's matmuls to run while the previous chunk's norm drains.
CHUNK_ORDER = [8, 0, 9, 1, 10, 2, 11, 3, 4, 5, 6, 7]


def _build_program(reps=1):
    """Build the single-core SPMD bass program. Returns (nc, out_name)."""
    nc = bacc.Bacc("TRN2", target_bir_lowering=False, debug=False,
                   num_devices=N_CORES)

    xT = nc.dram_tensor("xT", [D, S_ALL], MM_DT, kind="ExternalInput").ap()
    wqkv = nc.dram_tensor("wqkv", [D, 4 * H], MM_DT, kind="ExternalInput").ap()
    wo = nc.dram_tensor("wo", [HEADS_PER_CORE, H, D], MM_DT,
                        kind="ExternalInput").ap()
    # rope tables with the rms-norm scale vectors folded in:
    # [cA, sA, cB, sB] where xr1 = x1*cA - x2*sA ; xr2 = x2*cB + x1*sB
    ktab = nc.dram_tensor("ktab", [4, S_ALL, HALF], MM_DT,
                          kind="ExternalInput").ap()
    qtab = nc.dram_tensor("qtab", [4, T_NOISE, HALF], MM_DT,
                          kind="ExternalInput").ap()
    out = nc.dram_tensor("out", [T_NOISE, D], F32, kind="ExternalOutput").ap()

    with tile.TileContext(nc) as tc:
        for rep in range(reps):
            _emit(nc, tc, xT, wqkv, wo, ktab, qtab, out, pfx=f"r{rep}_")
    nc.compile()
    return nc, "out"


def _emit(nc, tc, xT, wqkv, wo, ktab, qtab, out, pfx=""):
    import contextlib
    ctx = contextlib.ExitStack()
    xTr = xT.rearrange("(d p) s -> p d s", p=P)
    with ctx:
        const = ctx.enter_context(tc.tile_pool(name=pfx + "const", bufs=1))
        persist = ctx.enter_context(tc.tile_pool(name=pfx + "persist", bufs=1))
        xp = ctx.enter_context(tc.tile_pool(name=pfx + "pa_x", bufs=3))

        # ---- weights + first x chunks first: PE starts ASAP ----
        wqkv_sb = const.tile([P, D_TILES * 4 * H], MM_DT, tag="wqkv")
        nc.sync.dma_start(wqkv_sb[:],
                          wqkv.rearrange("(d p) c -> p d c", p=P))
        prefetched = {}
        for sc in CHUNK_ORDER[:3]:
            xt = xp.tile([P, D_TILES * FREE], MM_DT, tag="xc", name="xchunk")
            nc.sync.dma_start(xt[:], xTr[:, :, sc * FREE:(sc + 1) * FREE])
            prefetched[sc] = xt

        # ---- remaining constants ----
        ident = const.tile([P, P], MM_DT, tag="ident")
        make_identity(nc, ident[:])
        ones = const.tile([P, 1], MM_DT, tag="ones")
        nc.vector.memset(ones[:], 1.0)
        eps_col = const.tile([P, 1], F32, tag="eps")
        nc.vector.memset(eps_col[:], EPS)
        ktab_sb = [persist.tile([P, S_TILES * HALF], MM_DT, tag=f"ktab{i}",
                                name=f"ktab{i}") for i in range(4)]
        qtab_sb = [persist.tile([P, T_TILES * HALF], MM_DT, tag=f"qtab{i}",
                                name=f"qtab{i}") for i in range(4)]
        # table/wo loads ride the ACT HWDGE ring so they never queue behind
        # the x-chunk prefetch stream on the SP ring
        for i in range(4):
            nc.scalar.dma_start(ktab_sb[i][:],
                                ktab[i].rearrange("(t p) f -> p t f", p=P))
            nc.scalar.dma_start(qtab_sb[i][:],
                                qtab[i].rearrange("(t p) f -> p t f", p=P))
        wo_sb = [const.tile([P, D], MM_DT, tag=f"wo{h}", name=f"wos{h}")
                 for h in range(HEADS_PER_CORE)]
        for h in range(HEADS_PER_CORE):
            nc.scalar.dma_start(wo_sb[h][:], wo[h])

        # ---- persistent activations ----
        kT_sb = persist.tile([P, S_ALL], MM_DT, tag="kT")
        v_sb = persist.tile([P, S_ALL], MM_DT, tag="v")     # [s-tile, h] blocks
        qT_sb = persist.tile([P, HEADS_PER_CORE * T_NOISE], MM_DT, tag="qT")
        oT_sb = persist.tile([P, HEADS_PER_CORE * T_NOISE], MM_DT, tag="oT")

        # ---- Phase A: fused QKV projection + norm/rope for all chunks ----
        with tc.tile_pool(name=pfx + "pa_ps", bufs=4, space="PSUM") as pa, \
             tc.tile_pool(name=pfx + "pa_ps2", bufs=2, space="PSUM") as pa2, \
             tc.tile_pool(name=pfx + "pa_pt", bufs=2, space="PSUM") as pst, \
             tc.tile_pool(name=pfx + "pa_w", bufs=4) as work, \
             tc.tile_pool(name=pfx + "pa_xr", bufs=6) as xrp:
            pending = []   # deferred transposes: (xr_row, [(j, dst), ...])

            def flush_pending():
                for xr_row, dsts in pending:
                    for j, dst in dsts:
                        pt = pst.tile([P, P], MM_DT, tag="pt")
                        nc.tensor.transpose(
                            pt[:], xr_row[:, j * P:(j + 1) * P], ident[:])
                        nc.vector.tensor_copy(dst, pt[:])
                pending.clear()

            def norm_rope_row(slots, off, tabs, tbase, rinv, rcol0, dsts):
                """Normalize+rope 4 token tiles (one 'row') at once.
                slots: 4 PSUM tiles; off: column offset of this row's H block;
                tabs: 4 table tiles; tbase: first table tile index;
                rinv: [P,12] stats tile, rcol0: first rinv column;
                dsts: 4 destination [P,128] SBUF slices (transposed)."""
                xn = xrp.tile([P, 4 * H], MM_DT, tag="xn")
                for j in range(4):
                    nc.scalar.activation(
                        xn[:, j * H:(j + 1) * H], slots[j][:, off:off + H],
                        mybir.ActivationFunctionType.Copy,
                        scale=rinv[:, rcol0 + j:rcol0 + j + 1])
                xnv = xn[:].rearrange("p (j h) -> p j h", j=4)
                x1 = xnv[:, :, 0:HALF]
                x2 = xnv[:, :, HALF:H]
                tsl = slice(tbase * HALF, (tbase + 4) * HALF)
                cA = tabs[0][:, tsl].rearrange("p (j f) -> p j f", j=4)
                sA = tabs[1][:, tsl].rearrange("p (j f) -> p j f", j=4)
                cB = tabs[2][:, tsl].rearrange("p (j f) -> p j f", j=4)
                sB = tabs[3][:, tsl].rearrange("p (j f) -> p j f", j=4)
                t1 = xrp.tile([P, 4 * HALF], MM_DT, tag="t1")
                t2 = xrp.tile([P, 4 * HALF], MM_DT, tag="t2")
                t1v = t1[:].rearrange("p (j f) -> p j f", j=4)
                t2v = t2[:].rearrange("p (j f) -> p j f", j=4)
                xr = xrp.tile([P, 4 * H], MM_DT, tag="xr")
                xrv = xr[:].rearrange("p (j h) -> p j h", j=4)
                nc.vector.tensor_mul(t1v, x1, cA)
                nc.vector.tensor_mul(t2v, x2, sA)
                nc.vector.tensor_sub(xrv[:, :, 0:HALF], t1v, t2v)
                nc.vector.tensor_mul(t1v, x2, cB)
                nc.vector.tensor_mul(t2v, x1, sB)
                nc.vector.tensor_add(xrv[:, :, HALF:H], t1v, t2v)
                pending.append((xr, dsts))

            for sc in CHUNK_ORDER:
                noise = sc >= NOISE_CHUNK0
                w_w = 4 * H if noise else 2 * H
                xt = prefetched.pop(sc, None)
                if xt is None:
                    xt = xp.tile([P, D_TILES * FREE], MM_DT, tag="xc",
                                 name="xchunk")
                    nc.sync.dma_start(
                        xt[:], xTr[:, :, sc * FREE:(sc + 1) * FREE])
                if noise:
                    slots = [pa.tile([P, w_w], F32, tag="pjn", name=f"pjn{j}")
                             for j in range(4)]
                else:
                    # pack two 256-wide ctx outputs per PSUM bank
                    banks = [pa2.tile([P, 2 * w_w], F32, tag="pjc",
                                      name=f"pjc{b}") for b in range(2)]
                    slots = [banks[j // 2][:, (j % 2) * w_w:(j % 2 + 1) * w_w]
                             for j in range(4)]
                # j outer, d inner: each j's accumulation group runs start..
                # stop without another group's start= clearing its bank's
                # has_written bits (two ctx groups share one PSUM bank).
                for j in range(4):
                    for d in range(D_TILES):
                        nc.tensor.matmul(
                            slots[j][:],
                            xt[:, d * FREE + j * P:d * FREE + (j + 1) * P],
                            wqkv_sb[:, d * 4 * H:d * 4 * H + w_w],
                            start=(d == 0), stop=(d == D_TILES - 1))
                flush_pending()   # prev chunk's transposes, inputs now ready
                # batched RMS stats: k per j (cols 0..3), q0 (4..7), q1 (8..11)
                ncols = 12 if noise else 4
                ssq = work.tile([P, 12], F32, tag="ssq")
                rms = work.tile([P, 12], F32, tag="rms")
                rinv = work.tile([P, 12], F32, tag="rinv")
                sq = [work.tile([P, H], F32, tag="sq", name=f"sq{j}")
                      for j in range(4)]
                for j in range(4):
                    nc.scalar.activation(
                        sq[j][:], slots[j][:, 0:H],
                        mybir.ActivationFunctionType.Square,
                        accum_out=ssq[:, j:j + 1])
                if noise:
                    for hh in range(HEADS_PER_CORE):
                        for j in range(4):
                            nc.scalar.activation(
                                sq[j][:], slots[j][:, (2 + hh) * H:(3 + hh) * H],
                                mybir.ActivationFunctionType.Square,
                                accum_out=ssq[:, 4 + 4 * hh + j:5 + 4 * hh + j])
                nc.scalar.activation(rms[:, 0:ncols], ssq[:, 0:ncols],
                                     mybir.ActivationFunctionType.Sqrt,
                                     bias=eps_col[:], scale=1.0 / H)
                nc.vector.reciprocal(rinv[:, 0:ncols], rms[:, 0:ncols])
                for j in range(4):
                    si = sc * 4 + j
                    nc.vector.tensor_copy(v_sb[:, si * P:(si + 1) * P],
                                          slots[j][:, H:2 * H])
                norm_rope_row(
                    slots, 0, ktab_sb, sc * 4, rinv, 0,
                    [(j, kT_sb[:, (sc * 4 + j) * P:(sc * 4 + j + 1) * P])
                     for j in range(4)])
                if noise:
                    ti0 = (sc - NOISE_CHUNK0) * 4
                    for hh in range(HEADS_PER_CORE):
                        norm_rope_row(
                            slots, (2 + hh) * H, qtab_sb, ti0, rinv,
                            4 + 4 * hh,
                            [(j, qT_sb[:, hh * T_NOISE + (ti0 + j) * P:
                                       hh * T_NOISE + (ti0 + j + 1) * P])
                             for j in range(4)])
            flush_pending()

        # ---- Phase C: attention (software-pipelined, quad row-sums) ----
        PAIR = 2 * FREE   # exp processes two score banks at once
        NP_ = S_TILES // 2
        NQ_ = S_TILES // 4
        with tc.tile_pool(name=pfx + "pc_sc", bufs=2, space="PSUM") as psc, \
             tc.tile_pool(name=pfx + "pc_av", bufs=2, space="PSUM") as pav, \
             tc.tile_pool(name=pfx + "pc_r", bufs=2, space="PSUM") as pr, \
             tc.tile_pool(name=pfx + "pc_exp", bufs=4) as pexp, \
             tc.tile_pool(name=pfx + "pc_w", bufs=2) as cwork:
            for hh in range(HEADS_PER_CORE):
                for tch in range(T_CHUNKS):
                    qslice = qT_sb[:, hh * T_NOISE + tch * FREE:
                                   hh * T_NOISE + (tch + 1) * FREE]
                    av = pav.tile([P, FREE], F32, tag="av")
                    rr = pr.tile([1, FREE], F32, tag="rr")
                    qpend = []
                    qstate = [0]

                    def scores_exp(sp):
                        sc_ps = psc.tile([P, PAIR], F32, tag="sc")
                        ex = pexp.tile([P, PAIR], MM_DT, tag="ex")
                        for u in range(2):
                            si = sp * 2 + u
                            nc.tensor.matmul(
                                sc_ps[:, u * FREE:(u + 1) * FREE],
                                kT_sb[:, si * P:(si + 1) * P], qslice,
                                start=True, stop=True)
                        nc.scalar.activation(ex[:], sc_ps[:],
                                             mybir.ActivationFunctionType.Exp,
                                             scale=INV_SQRT_H)
                        return ex

                    def av_mm(sp, ex):
                        for u in range(2):
                            si = sp * 2 + u
                            nc.tensor.matmul(
                                av[:], v_sb[:, si * P:(si + 1) * P],
                                ex[:, u * FREE:(u + 1) * FREE],
                                start=(si == 0), stop=(si == S_TILES - 1))
                        qpend.append(ex)
                        if len(qpend) == 2:
                            exA, exB = qpend
                            t0 = cwork.tile([P, FREE], MM_DT, tag="q0")
                            t1 = cwork.tile([P, FREE], MM_DT, tag="q1")
                            qs = cwork.tile([P, FREE], MM_DT, tag="qs")
                            nc.vector.tensor_add(t0[:], exA[:, 0:FREE],
                                                 exA[:, FREE:PAIR])
                            nc.vector.tensor_add(t1[:], exB[:, 0:FREE],
                                                 exB[:, FREE:PAIR])
                            nc.vector.tensor_add(qs[:], t0[:], t1[:])
                            qi = qstate[0]
                            nc.tensor.matmul(rr[:], ones[:], qs[:],
                                             start=(qi == 0),
                                             stop=(qi == NQ_ - 1))
                            qstate[0] = qi + 1
                            qpend.clear()

                    ex_prev = scores_exp(0)
                    for sp in range(1, NP_):
                        ex_cur = scores_exp(sp)
                        av_mm(sp - 1, ex_prev)
                        ex_prev = ex_cur
                    av_mm(NP_ - 1, ex_prev)

                    # normalize: oT = av / rowsum (broadcast recip over h)
                    cbase = hh * T_NOISE + tch * FREE
                    rrow = cwork.tile([1, FREE], F32, tag="rrow")
                    nc.vector.tensor_copy(rrow[:], rr[:])
                    rb = cwork.tile([P, FREE], F32, tag="rb")
                    nc.gpsimd.partition_broadcast(rb[:], rrow[:])
                    nc.vector.reciprocal(rb[:], rb[:])
                    nc.vector.tensor_mul(oT_sb[:, cbase:cbase + FREE],
                                         av[:], rb[:])

        # ---- Phase D: o-projection (2-head accumulate) + store ----
        with tc.tile_pool(name=pfx + "pd_ps", bufs=3, space="PSUM") as pso, \
             tc.tile_pool(name=pfx + "pd_o", bufs=3) as ostage:
            for ti in range(T_TILES):
                ot = ostage.tile([P, D], F32, tag="ot", name="otile")
                for dc in range(D_CHUNKS):
                    po = pso.tile([P, FREE], F32, tag="po")
                    for h in range(HEADS_PER_CORE):
                        nc.tensor.matmul(
                            po[:],
                            oT_sb[:, h * T_NOISE + ti * P:
                                  h * T_NOISE + (ti + 1) * P],
                            wo_sb[h][:, dc * FREE:(dc + 1) * FREE],
                            start=(h == 0), stop=(h == HEADS_PER_CORE - 1))
                    if dc % 2 == 0:
                        nc.vector.tensor_copy(
                            ot[:, dc * FREE:(dc + 1) * FREE], po[:])
                    else:
                        nc.scalar.activation(
                            ot[:, dc * FREE:(dc + 1) * FREE], po[:],
                            mybir.ActivationFunctionType.Copy)
                # alternate the two HWDGE rings so 1 MB stores overlap
                if ti % 2 == 0:
                    nc.sync.dma_start(out[ti * P:(ti + 1) * P, :], ot[:])
                else:
                    nc.scalar.dma_start(out[ti * P:(ti + 1) * P, :], ot[:])


def _get_program(reps=1):
    key = f"prog{reps}"
    if key not in _CACHE:
        _CACHE[key] = _build_program(reps)
    return _CACHE[key]


def prepare_in_maps(x_noise, target_hidden, Wq, Wk, Wv, Wo, q_scale, k_scale,
                    noise_positions, ctx_positions):
    x_noise = np.asarray(x_noise, dtype=np.float32)
    target_hidden = np.asarray(target_hidden, dtype=np.float32)
    Wq = np.asarray(Wq, dtype=np.float32)
    Wk = np.asarray(Wk, dtype=np.float32)
    Wv = np.asarray(Wv, dtype=np.float32)
    Wo = np.asarray(Wo, dtype=np.float32)
    q_scale = np.asarray(q_scale, dtype=np.float32)
    k_scale = np.asarray(k_scale, dtype=np.float32)

    x_all = np.concatenate([target_hidden, x_noise], axis=0)       # (S, D)
    xT = np.ascontiguousarray(x_all.T.astype(ml_dtypes.bfloat16))  # (D, S)
    pos_all = np.concatenate(
        [np.asarray(ctx_positions), np.asarray(noise_positions)]
    ).astype(np.float32)
    inv_freq = (ROPE_THETA ** (-np.arange(HALF, dtype=np.float32) * 2.0 / H)
                ).astype(np.float32)

    def rope_tabs(pos, scale):
        ang = pos[:, None] * inv_freq[None, :]
        co, sn = np.cos(ang), np.sin(ang)
        s1, s2 = scale[0:HALF], scale[HALF:H]
        return np.ascontiguousarray(np.stack(
            [co * s1, sn * s2, co * s2, sn * s1]
        ).astype(ml_dtypes.bfloat16))

    ktab = rope_tabs(pos_all, k_scale)                 # (4, S, 64)
    qtab = rope_tabs(pos_all[T_CTX:], q_scale)         # (4, T, 64)

    in_maps = []
    for c in range(N_CORES):
        wqkv = np.ascontiguousarray(np.concatenate(
            [Wk[:, c, :], Wv[:, c, :],
             Wq[:, 2 * c, :], Wq[:, 2 * c + 1, :]],
            axis=1).astype(ml_dtypes.bfloat16))                     # (D, 512)
        wo_c = np.ascontiguousarray(
            Wo[c * HEADS_PER_CORE:(c + 1) * HEADS_PER_CORE]
            .astype(ml_dtypes.bfloat16))                            # (2,128,D)
        in_maps.append({
            "xT": xT, "wqkv": wqkv, "wo": wo_c,
            "ktab": ktab, "qtab": qtab,
        })
    return in_maps


def kernel(**inputs):
    in_maps = prepare_in_maps(**inputs)
    nc, out_name = _get_program()
    res = run_bass_kernel_spmd(nc, in_maps, core_ids=list(range(N_CORES)))
    acc = np.zeros((T_NOISE, D), dtype=np.float32)
    for r in res.results:
        acc += r[out_name]
    return acc


def run_traced(inputs, **kw):
    """Run once with NTFF tracing; returns BassKernelResults (exec_time_ns)."""
    in_maps = prepare_in_maps(**inputs)
    nc, out_name = _get_program()
    return run_bass_kernel_spmd(nc, in_maps, core_ids=list(range(N_CORES)),
                                trace=True, **kw)
